# revision 10
# baseline (speedup 1.0000x reference)
"""CrossAttention kernel for Trainium2, 8 NeuronCores.

Reference pipeline (B=4, C=256, H=W=64, N=4096, d=C//8=32):
  sub = x1 - x2
  x3 = relu(bn1(pw1(dw1([sub, x1]))))      # dw: 3x3 grouped conv (groups=C)
  x4 = relu(bn2(pw2(dw2([sub, x2]))))      # pw: 1x1 512->256
  q = wq@x4 [B,32,N]; k = wk@x3 [B,32,N]; v = wv@x3 [B,256,N]
  attn = softmax(q^T k);  out = gamma * (v @ attn^T) + x1

Sharding: 8 cores = (batch b) x (pixel-half h). Each core computes BOTH
conv paths only for its own pixel half (with a one-row halo), projects
k / v^T / q from its half, then the pair exchanges k and v^T via
AllGather so each core can run flash attention for its 2048 queries over
all 4096 keys.

Device-side choices:
  - dw conv on the PE as 9 accumulating block-diagonal [128x128] matmuls
    over a zero-padded 66-col image layout (8-row windows = 512 output
    columns via a strided rhs AP that skips the pad columns).
  - energy is computed transposed, E^T[j, i]; the softmax denominator
    comes for free from an appended ones-column in v^T in the second
    (accumulating) matmul.
  - matmul datapath in bf16 (fp32 PSUM accumulation); normalize /
    transpose / residual-add in fp32.
  - gamma folded into wv/bv on the host; bn+biases folded into per-
    channel scale/shift applied by ScalarE during PSUM eviction.
"""

import numpy as np
import ml_dtypes

import concourse.bass as bass
import concourse.mybir as mybir
import concourse.tile as tile
from concourse import bacc
from concourse.bass_utils import run_bass_kernel_spmd

F32 = mybir.dt.float32
BF16 = mybir.dt.bfloat16
AF = mybir.ActivationFunctionType
ALU = mybir.AluOpType

B, C, H, W = 4, 256, 64, 64
N = H * W            # 4096 pixels
QH = N // 2          # pixels per core (queries/own keys)
EPS = 1e-5
PW = 66              # padded row width
OFF = 2              # leading pad elements in padded tiles
SLOTS = 34           # 32 data rows + halo/zero rows
CAT_F = OFF + SLOTS * PW + OFF   # 2248
VT = 258             # v^T row: 256 channels + ones + pad
PAIRS = [[0, 1], [2, 3], [4, 5], [6, 7]]

_CACHE = {}


def _build_nc():
    nc = bacc.Bacc("TRN2", target_bir_lowering=False, debug=False, num_devices=8)

    cat1p = nc.dram_tensor("cat1p", [4, 128, CAT_F], BF16, kind="ExternalInput")
    cat2p = nc.dram_tensor("cat2p", [4, 128, CAT_F], BF16, kind="ExternalInput")
    x1h_d = nc.dram_tensor("x1h", [2, 128, QH], F32, kind="ExternalInput")
    w1bd = nc.dram_tensor("w1bd", [4, 128, 9 * 128], BF16, kind="ExternalInput")
    w2bd = nc.dram_tensor("w2bd", [4, 128, 9 * 128], BF16, kind="ExternalInput")
    pw1T = nc.dram_tensor("pw1T", [4, 128, 256], BF16, kind="ExternalInput")
    pw2T = nc.dram_tensor("pw2T", [4, 128, 256], BF16, kind="ExternalInput")
    wvT = nc.dram_tensor("wvT", [2, 128, 256], BF16, kind="ExternalInput")
    wkT = nc.dram_tensor("wkT", [2, 128, 32], BF16, kind="ExternalInput")
    wqT = nc.dram_tensor("wqT", [2, 128, 32], BF16, kind="ExternalInput")
    bn1_d = nc.dram_tensor("bn1", [128, 4], F32, kind="ExternalInput")
    bn2_d = nc.dram_tensor("bn2", [128, 4], F32, kind="ExternalInput")
    bkq_d = nc.dram_tensor("bkq", [128, 1], F32, kind="ExternalInput")
    bvg_d = nc.dram_tensor("bvg", [2, 128, 1], F32, kind="ExternalInput")
    vinit_d = nc.dram_tensor("vinit", [128, 16 * VT], BF16, kind="ExternalInput")
    out_d = nc.dram_tensor("out", [2, 128, QH], F32, kind="ExternalOutput")

    # collective bounce buffers: one flat gather of v^T (128x4128) + k (32x2048)
    KVN = 128 * 16 * VT + 32 * QH   # 593920
    kvown_d = nc.dram_tensor("kvown_b", [KVN], BF16)
    kvfull_d = nc.dram_tensor("kvfull_b", [2 * KVN], BF16)

    with tile.TileContext(nc) as tc:
        with tc.tile_pool(name="persist", bufs=1) as pp:
            x3o = [pp.tile([128, QH], BF16, name=f"x3o_{m}", tag=f"x3o_{m}")
                   for m in range(2)]
            x4 = [pp.tile([128, QH], BF16, name=f"x4_{m}", tag=f"x4_{m}")
                  for m in range(2)]
            bn1 = pp.tile([128, 4], F32, name="bn1", tag="bn1")
            bn2 = pp.tile([128, 4], F32, name="bn2", tag="bn2")
            nc.sync.dma_start(bn1[:], bn1_d[:])
            nc.sync.dma_start(bn2[:], bn2_d[:])

            # prefetch both conv blocks' inputs and weights up front so
            # the pair AllGather later never contends with input DMA traffic
            cats = [[pp.tile([128, CAT_F], BF16, name=f"cat{b}_{k}",
                             tag=f"cat{b}_{k}") for k in range(4)]
                    for b in range(2)]
            wsbs = [[pp.tile([128, 9 * 128], BF16, name=f"wbd{b}_{k}",
                             tag=f"wbd{b}_{k}") for k in range(4)]
                    for b in range(2)]
            pwsbs = [[pp.tile([128, 256], BF16, name=f"pwT{b}_{k}",
                              tag=f"pwT{b}_{k}") for k in range(4)]
                     for b in range(2)]
            # persistent attention-side tiles, allocated before any transient
            # pool so no false WAR deps delay their fills
            k_own = pp.tile([128, QH], BF16, name="k_own", tag="k_own")
            vto = pp.tile([128, 16 * VT], BF16, name="vto", tag="vto")
            k_sb = pp.tile([128, N], BF16, name="k_sb", tag="k_sb")
            q_sb = pp.tile([128, QH], BF16, name="q_sb", tag="q_sb")
            vta = pp.tile([128, 32 * VT], BF16, name="vta", tag="vta")
            x1h = [pp.tile([128, QH], F32, name=f"x1h_{c}", tag=f"x1h_{c}")
                   for c in range(2)]
            out_sb = [pp.tile([128, QH], F32, name=f"osb_{c}", tag=f"osb_{c}")
                      for c in range(2)]
            ones_sb = pp.tile([128, 128], BF16, name="ones", tag="ones")
            bkq = pp.tile([128, 1], F32, name="bkq", tag="bkq")
            bvg = pp.tile([128, 2], F32, name="bvg", tag="bvg")
            wv_sb = [pp.tile([128, 256], BF16, name=f"wv_{c}", tag=f"wv_{c}")
                     for c in range(2)]
            wk_sb = [pp.tile([128, 32], BF16, name=f"wk_{c}", tag=f"wk_{c}")
                     for c in range(2)]
            wq_sb = [pp.tile([128, 32], BF16, name=f"wq_{c}", tag=f"wq_{c}")
                     for c in range(2)]
            # zero rows 32:128 so energy matmuls can use full 128-row lhsT
            # (avoids the PE small-tile row-group slowdown)
            nc.gpsimd.memset(k_sb[:], 0.0)
            nc.gpsimd.memset(q_sb[:], 0.0)
            nc.gpsimd.memset(ones_sb[:], 1.0)
            nc.sync.dma_start(bkq[:], bkq_d[:])
            for ch in range(2):
                nc.sync.dma_start(bvg[:, ch:ch + 1], bvg_d[ch])
                nc.sync.dma_start(wv_sb[ch][:], wvT[ch])
                nc.sync.dma_start(wk_sb[ch][:], wkT[ch])
                nc.sync.dma_start(wq_sb[ch][:], wqT[ch])

            # input DMAs in priority order: conv1 first, then v^T init +
            # conv2, then the residual halves (needed only at the end)
            for k in range(4):
                nc.sync.dma_start(cats[0][k][:], cat1p[k])
                nc.sync.dma_start(wsbs[0][k][:], w1bd[k])
                nc.sync.dma_start(pwsbs[0][k][:], pw1T[k])
            nc.sync.dma_start(vto[:], vinit_d[:])
            for k in range(4):
                nc.sync.dma_start(cats[1][k][:], cat2p[k])
                nc.sync.dma_start(wsbs[1][k][:], w2bd[k])
                nc.sync.dma_start(pwsbs[1][k][:], pw2T[k])
            for ch in range(2):
                nc.sync.dma_start(x1h[ch][:], x1h_d[ch])

            def conv_block(cat_sb, w_sb, pw_sb, bn, xout):
                with tc.tile_pool(name="conv_y", bufs=2) as cyb, \
                     tc.tile_pool(name="conv_ps", bufs=2, space="PSUM") as cps:
                    for w in range(4):
                        y1w = [cyb.tile([128, 512], BF16,
                                        name=f"y1w_{k}", tag=f"y1w_{k}")
                               for k in range(4)]
                        for k in range(4):
                            ps = cps.tile([128, 512], F32, name="dwps", tag="dwps")
                            for t in range(9):
                                dr, dc = t // 3, t % 3
                                start = OFF + (8 * w + dr) * PW + dc - 1
                                rhs = cat_sb[k][:, start:start + 8 * PW] \
                                    .rearrange("p (r c) -> p r c", r=8, c=PW)[:, :, 0:64]
                                nc.tensor.matmul(
                                    ps[:], w_sb[k][:, 128 * t:128 * (t + 1)], rhs,
                                    start=(t == 0), stop=(t == 8))
                            nc.scalar.activation(y1w[k][:], ps[:], AF.Copy)
                        for m in range(2):
                            pp2 = cps.tile([128, 512], F32, name="pwps", tag="pwps")
                            for k in range(4):
                                nc.tensor.matmul(
                                    pp2[:], pw_sb[k][:, 128 * m:128 * (m + 1)],
                                    y1w[k][:], start=(k == 0), stop=(k == 3))
                            nc.scalar.activation(
                                xout[m][:, 512 * w:512 * (w + 1)], pp2[:],
                                AF.Relu, bias=bn[:, 2 * m + 1:2 * m + 2],
                                scale=bn[:, 2 * m:2 * m + 1])

            conv_block(cats[0], wsbs[0], pwsbs[0], bn1, x3o)

            # ---- own-half projections: k_own, v^T_own ----
            with tc.tile_pool(name="proj_ps", bufs=2, space="PSUM") as pps:
                for s in range(4):
                    ps = pps.tile([128, 512], F32, name="kqps", tag="kqps")
                    for ch in range(2):
                        nc.tensor.matmul(ps[0:32, :], wk_sb[ch][:],
                                         x3o[ch][:, 512 * s:512 * (s + 1)],
                                         start=(ch == 0), stop=(ch == 1))
                    nc.scalar.activation(k_own[0:32, 512 * s:512 * (s + 1)],
                                         ps[0:32, :], AF.Identity,
                                         bias=bkq[0:32, 0:1])
                for j in range(16):
                    ps = pps.tile([128, 256], F32, name="vtps", tag="vtps")
                    for ch in range(2):
                        nc.tensor.matmul(ps[:], x3o[ch][:, 128 * j:128 * (j + 1)],
                                         wv_sb[ch][:], start=(ch == 0), stop=(ch == 1))
                    nc.scalar.activation(vto[:, VT * j:VT * j + 256], ps[:], AF.Copy)

                # ship own k / v^T, single AllGather for the pair
                VSZ = 128 * 16 * VT
                nc.sync.dma_start(
                    kvown_d[0:VSZ].rearrange("(p f) -> p f", p=128), vto[:])
                nc.sync.dma_start(
                    kvown_d[VSZ:KVN].rearrange("(p f) -> p f", p=32),
                    k_own[0:32, :])
                nc.gpsimd.collective_compute(
                    "AllGather", ALU.bypass, replica_groups=PAIRS,
                    ins=[kvown_d[:].opt()], outs=[kvfull_d[:].opt()])
                for m in range(2):
                    o = m * KVN
                    nc.sync.dma_start(
                        vta[:, 16 * VT * m:16 * VT * (m + 1)],
                        kvfull_d[o:o + VSZ].rearrange("(p f) -> p f", p=128))
                    nc.sync.dma_start(
                        k_sb[0:32, QH * m:QH * (m + 1)],
                        kvfull_d[o + VSZ:o + KVN].rearrange("(p f) -> p f",
                                                            p=32))

                # conv2 + q overlap with the collective
                conv_block(cats[1], wsbs[1], pwsbs[1], bn2, x4)
                for s in range(4):
                    ps = pps.tile([128, 512], F32, name="kqps", tag="kqps")
                    for ch in range(2):
                        nc.tensor.matmul(ps[0:32, :], wq_sb[ch][:],
                                         x4[ch][:, 512 * s:512 * (s + 1)],
                                         start=(ch == 0), stop=(ch == 1))
                    nc.scalar.activation(q_sb[0:32, 512 * s:512 * (s + 1)],
                                         ps[0:32, :], AF.Identity,
                                         bias=bkq[32:64, 0:1])


            # ---- flash attention (output computed pre-transposed) ----
            # acc_c[c, i] = sum_j v^T[j, c] * ex[j, i]  via lhsT=vta block,
            # rhs=ex: 512-wide matmuls, output lands as [channel, query] so
            # no PE transposes are needed. The softmax denominator comes
            # from an all-ones lhsT matmul, which also broadcasts it across
            # all 128 partitions for the DVE normalize.
            with tc.tile_pool(name="att_sb", bufs=2) as asb, \
                 tc.tile_pool(name="acc_ps", bufs=2, space="PSUM") as accp, \
                 tc.tile_pool(name="e_ps", bufs=2, space="PSUM") as epsp:
                for ib in range(4):
                    acc = [accp.tile([128, 512], F32, name=f"acc{c}",
                                     tag=f"acc{c}") for c in range(2)]
                    dps = accp.tile([128, 512], F32, name="dps", tag="dps")
                    eps_t = {}
                    for j in range(32):
                        if j == 0:
                            eps_t[0] = epsp.tile([128, 512], F32, name="eps",
                                                 tag="eps")
                            nc.tensor.matmul(eps_t[0][:],
                                             k_sb[:, 0:128],
                                             q_sb[:, 512 * ib:512 * (ib + 1)],
                                             start=True, stop=True)
                        ex = asb.tile([128, 512], BF16, name="ex", tag="ex")
                        nc.scalar.activation(ex[:], eps_t[j][:], AF.Exp)
                        if j + 1 < 32:
                            eps_t[j + 1] = epsp.tile([128, 512], F32, name="eps",
                                                     tag="eps")
                            nc.tensor.matmul(eps_t[j + 1][:],
                                             k_sb[:, 128 * (j + 1):128 * (j + 2)],
                                             q_sb[:, 512 * ib:512 * (ib + 1)],
                                             start=True, stop=True)
                        eps_t.pop(j - 1, None)
                        nc.tensor.matmul(acc[0][:],
                                         vta[:, VT * j:VT * j + 128], ex[:],
                                         start=(j == 0), stop=(j == 31))
                        nc.tensor.matmul(acc[1][:],
                                         vta[:, VT * j + 128:VT * j + 256], ex[:],
                                         start=(j == 0), stop=(j == 31))
                        nc.tensor.matmul(dps[:], ones_sb[:], ex[:],
                                             start=(j == 0), stop=(j == 31))
                    rec = asb.tile([128, 512], F32, name="rec", tag="rec")
                    nc.vector.reciprocal(rec[:], dps[:])
                    for ch in range(2):
                        tmp = asb.tile([128, 512], F32, name="tmp", tag="tmp")
                        nc.vector.scalar_tensor_tensor(
                            tmp[:], acc[ch][:], 1.0, rec[:],
                            ALU.mult, ALU.mult)
                        nc.vector.scalar_tensor_tensor(
                            out_sb[ch][:, 512 * ib:512 * (ib + 1)], tmp[:],
                            bvg[:, ch:ch + 1],
                            x1h[ch][:, 512 * ib:512 * (ib + 1)],
                            ALU.add, ALU.add)
                        nc.sync.dma_start(
                            out_d[ch][:, 512 * ib:512 * (ib + 1)],
                            out_sb[ch][:, 512 * ib:512 * (ib + 1)])
    nc.compile()
    return nc


def _prep_shared(inputs):
    f = np.float32
    bf = ml_dtypes.bfloat16

    def bd(w_dw):
        wr = w_dw.reshape(512, 2, 9)
        Wt = np.zeros((4, 128, 9, 128), f)
        m = np.arange(64)
        for k in range(4):
            blk = wr[128 * k:128 * (k + 1)]        # [128, 2, 9]
            for i in range(2):
                for j in range(2):
                    Wt[k, 2 * m + i, :, 2 * m + j] = blk[2 * m + j, i, :]
        return np.ascontiguousarray(Wt.reshape(4, 128, 9 * 128)).astype(bf)

    w1bd = bd(inputs["w1_dw"])
    w2bd = bd(inputs["w2_dw"])

    pw1 = inputs["w1_pw"][:, :, 0, 0]              # [256, 512]
    pw2 = inputs["w2_pw"][:, :, 0, 0]
    pw1T = np.ascontiguousarray(pw1.T.reshape(4, 128, 256)).astype(bf)
    pw2T = np.ascontiguousarray(pw2.T.reshape(4, 128, 256)).astype(bf)

    gamma = float(inputs["gamma"][0])
    wvTg = np.ascontiguousarray(
        (inputs["wv"][:, :, 0, 0].T * gamma).reshape(2, 128, 256).astype(bf))
    wkT = np.ascontiguousarray(
        inputs["wk"][:, :, 0, 0].T.reshape(2, 128, 32)).astype(bf)
    wqT = np.ascontiguousarray(
        inputs["wq"][:, :, 0, 0].T.reshape(2, 128, 32)).astype(bf)

    def bn_fold(g, b_, mean, var, pw, b_dw, b_pw):
        s = g / np.sqrt(var + EPS)
        bc = pw @ b_dw + b_pw
        t = s * (bc - mean) + b_
        o = np.zeros((128, 4), f)
        o[:, 0], o[:, 1] = s[0:128], t[0:128]
        o[:, 2], o[:, 3] = s[128:256], t[128:256]
        return o

    bn1 = bn_fold(inputs["bn1_g"], inputs["bn1_b"], inputs["bn1_m"],
                  inputs["bn1_v"], pw1, inputs["b1_dw"], inputs["b1_pw"])
    bn2 = bn_fold(inputs["bn2_g"], inputs["bn2_b"], inputs["bn2_m"],
                  inputs["bn2_v"], pw2, inputs["b2_dw"], inputs["b2_pw"])

    bkq = np.zeros((128, 1), f)
    bkq[0:32, 0] = inputs["bk"]
    bkq[32:64, 0] = inputs["bq"]
    bvg = np.ascontiguousarray((gamma * inputs["bv"]).reshape(2, 128, 1).astype(f))

    vinit = np.zeros((128, 16 * VT), bf)
    for j in range(16):
        vinit[:, VT * j + 256] = 1.0

    return dict(w1bd=w1bd, w2bd=w2bd, pw1T=pw1T, pw2T=pw2T, wvT=wvTg,
                wkT=wkT, wqT=wqT, bn1=bn1, bn2=bn2, bkq=bkq, bvg=bvg,
                vinit=vinit)


def _prep_core(inputs, b, h):
    bf = ml_dtypes.bfloat16
    x1 = inputs["x1"][b]          # [256, 64, 64]
    x2 = inputs["x2"][b]
    sub = x1 - x2
    cat1 = np.concatenate([sub, x1], axis=0).reshape(4, 128, 64, 64)
    cat2 = np.concatenate([sub, x2], axis=0).reshape(4, 128, 64, 64)

    def pad_half(cc):
        buf = np.zeros((4, 128, SLOTS, 66), np.float32)
        if h == 0:
            buf[:, :, 1:34, 1:65] = cc[:, :, 0:33, :]
        else:
            buf[:, :, 0:33, 1:65] = cc[:, :, 31:64, :]
        catp = np.zeros((4, 128, CAT_F), bf)
        catp[:, :, OFF:OFF + SLOTS * PW] = buf.reshape(4, 128, -1)
        return catp

    x1h = np.ascontiguousarray(
        x1.reshape(256, N)[:, QH * h:QH * (h + 1)].reshape(2, 128, QH))
    return dict(cat1p=pad_half(cat1), cat2p=pad_half(cat2), x1h=x1h)


def kernel(**inputs):
    if "nc" not in _CACHE:
        _CACHE["nc"] = _build_nc()
    nc = _CACHE["nc"]

    inputs = {k: np.ascontiguousarray(np.asarray(v)) for k, v in inputs.items()}
    shared = _prep_shared(inputs)
    in_maps = []
    for core in range(8):
        b, h = core // 2, core % 2
        m = dict(shared)
        m.update(_prep_core(inputs, b, h))
        in_maps.append(m)

    res = run_bass_kernel_spmd(nc, in_maps, list(range(8)))
    out = np.empty((4, 256, N), np.float32)
    for core in range(8):
        b, h = core // 2, core % 2
        r = res.results[core]["out"]
        out[b, 0:128, QH * h:QH * (h + 1)] = r[0]
        out[b, 128:256, QH * h:QH * (h + 1)] = r[1]
    return out.reshape(B, C, H, W)



# revision 11
# speedup vs baseline: 1.1648x; 1.1648x over previous
"""CrossAttention kernel for Trainium2, 8 NeuronCores.

Reference pipeline (B=4, C=256, H=W=64, N=4096, d=C//8=32):
  sub = x1 - x2
  x3 = relu(bn1(pw1(dw1([sub, x1]))))      # dw: 3x3 grouped conv (groups=C)
  x4 = relu(bn2(pw2(dw2([sub, x2]))))      # pw: 1x1 512->256
  q = wq@x4 [B,32,N]; k = wk@x3 [B,32,N]; v = wv@x3 [B,256,N]
  attn = softmax(q^T k);  out = gamma * (v @ attn^T) + x1

Sharding: 8 cores = (batch b) x (pixel-half h). Each core computes BOTH
conv paths only for its own pixel half (with a one-row halo), projects
k / v^T / q from its half, then the pair exchanges k and v^T via a
single AllGather so each core runs flash attention for its 2048 queries
over all 4096 keys.

Device-side choices:
  - dw conv on the PE as 9 accumulating block-diagonal [128x128] matmuls
    over a zero-padded 66-col image layout (8-row windows = 512 output
    columns via a strided rhs AP that skips the pad columns).
  - energy is computed transposed, E^T[j, i], with k/q zero-padded to a
    128-deep contraction so the PE streams at full rate.
  - attention accumulation outputs [channel, query] directly (lhsT =
    v^T block, rhs = exp tile): 512-wide matmuls, no PE transposes. The
    softmax denominator comes from an all-ones lhsT matmul which also
    broadcasts it across partitions for the DVE normalize.
  - all inputs packed into few large DMAs (descriptor generation on the
    sync engine costs ~0.7us per dma_start, so count matters).
  - matmul datapath in bf16 (fp32 PSUM accumulation); normalize /
    residual-add in fp32. gamma folded into wv/bv on the host; bn+biases
    folded into per-channel scale/shift applied during PSUM eviction.
"""

import numpy as np
import ml_dtypes

import concourse.bass as bass
import concourse.mybir as mybir
import concourse.tile as tile
from concourse import bacc
from concourse.bass_utils import run_bass_kernel_spmd

F32 = mybir.dt.float32
BF16 = mybir.dt.bfloat16
AF = mybir.ActivationFunctionType
ALU = mybir.AluOpType

B, C, H, W = 4, 256, 64, 64
N = H * W            # 4096 pixels
QH = N // 2          # pixels per core (queries/own keys)
EPS = 1e-5
PW = 66              # padded row width
OFF = 2              # leading pad elements in padded tiles
SLOTS = 34           # 32 data rows + halo/zero rows
CAT_F = OFF + SLOTS * PW + OFF   # 2248
CATW = CAT_F + 9 * 128 + 256     # cat | dw weights | pw weights = 3656
VT = 258             # v^T row stride: 256 channels + 2 pad
PAIRS = [[0, 1], [2, 3], [4, 5], [6, 7]]

_CACHE = {}


def _build_nc():
    nc = bacc.Bacc("TRN2", target_bir_lowering=False, debug=False, num_devices=8)

    c1w_d = nc.dram_tensor("c1w", [4, 128, CATW], BF16, kind="ExternalInput")
    c2w_d = nc.dram_tensor("c2w", [4, 128, CATW], BF16, kind="ExternalInput")
    x1h_d = nc.dram_tensor("x1h", [128, 2 * QH], F32, kind="ExternalInput")
    consts_d = nc.dram_tensor("consts", [128, 11], F32, kind="ExternalInput")
    projw_d = nc.dram_tensor("projw", [128, 640], BF16, kind="ExternalInput")
    out_d = nc.dram_tensor("out", [128, 2 * QH], F32, kind="ExternalOutput")

    # collective bounce buffers: one flat gather of v^T (128x4128) + k (32x2048)
    KVN = 128 * 16 * VT + 32 * QH   # 593920
    VSZ = 128 * 16 * VT
    kvown_d = nc.dram_tensor("kvown_b", [KVN], BF16)
    kvfull_d = nc.dram_tensor("kvfull_b", [2 * KVN], BF16)

    with tile.TileContext(nc) as tc:
        with tc.tile_pool(name="persist", bufs=1) as pp:
            catw = [[pp.tile([128, CATW], BF16, name=f"cw{b}_{k}",
                             tag=f"cw{b}_{k}") for k in range(4)]
                    for b in range(2)]
            x3o = [pp.tile([128, QH], BF16, name=f"x3o_{m}", tag=f"x3o_{m}")
                   for m in range(2)]
            x4 = [pp.tile([128, QH], BF16, name=f"x4_{m}", tag=f"x4_{m}")
                  for m in range(2)]
            consts = pp.tile([128, 11], F32, name="consts", tag="consts")
            projw = pp.tile([128, 640], BF16, name="projw", tag="projw")
            k_own = pp.tile([128, QH], BF16, name="k_own", tag="k_own")
            vto = pp.tile([128, 16 * VT], BF16, name="vto", tag="vto")
            k_sb = pp.tile([128, N], BF16, name="k_sb", tag="k_sb")
            q_sb = pp.tile([128, QH], BF16, name="q_sb", tag="q_sb")
            vta = pp.tile([128, 32 * VT], BF16, name="vta", tag="vta")
            x1h = pp.tile([128, 2 * QH], F32, name="x1h", tag="x1h")
            out_sb = pp.tile([128, 2 * QH], F32, name="osb", tag="osb")
            ones_sb = pp.tile([128, 128], BF16, name="ones", tag="ones")
            # zero rows 32:128 so energy matmuls can use full 128-row lhsT
            # (avoids the PE small-tile row-group slowdown)
            nc.gpsimd.memset(k_sb[:], 0.0)
            nc.gpsimd.memset(q_sb[:], 0.0)
            nc.gpsimd.memset(ones_sb[:], 1.0)

            # input DMAs in priority order (conv1, consts, conv2, residual)
            for k in range(4):
                nc.sync.dma_start(catw[0][k][:], c1w_d[k])
            nc.sync.dma_start(consts[:], consts_d[:])
            nc.sync.dma_start(projw[:], projw_d[:])
            for k in range(4):
                nc.sync.dma_start(catw[1][k][:], c2w_d[k])
            nc.sync.dma_start(x1h[:], x1h_d[:])

            def conv_block(cw, bno, xout):
                with tc.tile_pool(name="conv_y", bufs=2) as cyb, \
                     tc.tile_pool(name="conv_ps", bufs=2, space="PSUM") as cps:
                    for w in range(4):
                        y1w = [cyb.tile([128, 512], BF16,
                                        name=f"y1w_{k}", tag=f"y1w_{k}")
                               for k in range(4)]
                        for k in range(4):
                            ps = cps.tile([128, 512], F32, name="dwps", tag="dwps")
                            for t in range(9):
                                dr, dc = t // 3, t % 3
                                start = OFF + (8 * w + dr) * PW + dc - 1
                                rhs = cw[k][:, start:start + 8 * PW] \
                                    .rearrange("p (r c) -> p r c", r=8, c=PW)[:, :, 0:64]
                                nc.tensor.matmul(
                                    ps[:],
                                    cw[k][:, CAT_F + 128 * t:CAT_F + 128 * (t + 1)],
                                    rhs, start=(t == 0), stop=(t == 8))
                            nc.scalar.activation(y1w[k][:], ps[:], AF.Copy)
                        for m in range(2):
                            pp2 = cps.tile([128, 512], F32, name="pwps", tag="pwps")
                            pwo = CAT_F + 9 * 128
                            for k in range(4):
                                nc.tensor.matmul(
                                    pp2[:],
                                    cw[k][:, pwo + 128 * m:pwo + 128 * (m + 1)],
                                    y1w[k][:], start=(k == 0), stop=(k == 3))
                            nc.scalar.activation(
                                xout[m][:, 512 * w:512 * (w + 1)], pp2[:],
                                AF.Relu,
                                bias=consts[:, bno + 2 * m + 1:bno + 2 * m + 2],
                                scale=consts[:, bno + 2 * m:bno + 2 * m + 1])

            conv_block(catw[0], 0, x3o)

            # ---- own-half projections: k_own, v^T_own ----
            with tc.tile_pool(name="proj_ps", bufs=2, space="PSUM") as pps:
                for s in range(4):
                    ps = pps.tile([128, 512], F32, name="kqps", tag="kqps")
                    for ch in range(2):
                        nc.tensor.matmul(ps[0:32, :],
                                         projw[:, 320 * ch + 256:320 * ch + 288],
                                         x3o[ch][:, 512 * s:512 * (s + 1)],
                                         start=(ch == 0), stop=(ch == 1))
                    nc.scalar.activation(k_own[0:32, 512 * s:512 * (s + 1)],
                                         ps[0:32, :], AF.Identity,
                                         bias=consts[0:32, 8:9])
                for j in range(16):
                    ps = pps.tile([128, 256], F32, name="vtps", tag="vtps")
                    for ch in range(2):
                        nc.tensor.matmul(ps[:], x3o[ch][:, 128 * j:128 * (j + 1)],
                                         projw[:, 320 * ch:320 * ch + 256],
                                         start=(ch == 0), stop=(ch == 1))
                    nc.scalar.activation(vto[:, VT * j:VT * j + 256], ps[:], AF.Copy)

                # ship own k / v^T, single AllGather for the pair
                nc.sync.dma_start(
                    kvown_d[0:VSZ].rearrange("(p f) -> p f", p=128), vto[:])
                nc.sync.dma_start(
                    kvown_d[VSZ:KVN].rearrange("(p f) -> p f", p=32),
                    k_own[0:32, :])
                nc.gpsimd.collective_compute(
                    "AllGather", ALU.bypass, replica_groups=PAIRS,
                    ins=[kvown_d[:].opt()], outs=[kvfull_d[:].opt()])
                for m in range(2):
                    o = m * KVN
                    nc.sync.dma_start(
                        vta[:, 16 * VT * m:16 * VT * (m + 1)],
                        kvfull_d[o:o + VSZ].rearrange("(p f) -> p f", p=128))
                    nc.sync.dma_start(
                        k_sb[0:32, QH * m:QH * (m + 1)],
                        kvfull_d[o + VSZ:o + KVN].rearrange("(p f) -> p f",
                                                            p=32))

                # conv2 + q overlap with the collective
                conv_block(catw[1], 4, x4)
                for s in range(4):
                    ps = pps.tile([128, 512], F32, name="kqps", tag="kqps")
                    for ch in range(2):
                        nc.tensor.matmul(ps[0:32, :],
                                         projw[:, 320 * ch + 288:320 * ch + 320],
                                         x4[ch][:, 512 * s:512 * (s + 1)],
                                         start=(ch == 0), stop=(ch == 1))
                    nc.scalar.activation(q_sb[0:32, 512 * s:512 * (s + 1)],
                                         ps[0:32, :], AF.Identity,
                                         bias=consts[32:64, 8:9])

            # ---- flash attention (output computed pre-transposed) ----
            with tc.tile_pool(name="att_sb", bufs=2) as asb, \
                 tc.tile_pool(name="acc_ps", bufs=2, space="PSUM") as accp, \
                 tc.tile_pool(name="e_ps", bufs=2, space="PSUM") as epsp:
                for ib in range(4):
                    acc = [accp.tile([128, 512], F32, name=f"acc{c}",
                                     tag=f"acc{c}") for c in range(2)]
                    dps = accp.tile([128, 512], F32, name="dps", tag="dps")
                    eps_t = {}
                    for j in range(32):
                        if j == 0:
                            eps_t[0] = epsp.tile([128, 512], F32, name="eps",
                                                 tag="eps")
                            nc.tensor.matmul(eps_t[0][:],
                                             k_sb[:, 0:128],
                                             q_sb[:, 512 * ib:512 * (ib + 1)],
                                             start=True, stop=True)
                        ex = asb.tile([128, 512], BF16, name="ex", tag="ex")
                        nc.scalar.activation(ex[:], eps_t[j][:], AF.Exp)
                        if j + 1 < 32:
                            eps_t[j + 1] = epsp.tile([128, 512], F32, name="eps",
                                                     tag="eps")
                            nc.tensor.matmul(eps_t[j + 1][:],
                                             k_sb[:, 128 * (j + 1):128 * (j + 2)],
                                             q_sb[:, 512 * ib:512 * (ib + 1)],
                                             start=True, stop=True)
                        eps_t.pop(j - 1, None)
                        nc.tensor.matmul(acc[0][:],
                                         vta[:, VT * j:VT * j + 128], ex[:],
                                         start=(j == 0), stop=(j == 31))
                        nc.tensor.matmul(acc[1][:],
                                         vta[:, VT * j + 128:VT * j + 256], ex[:],
                                         start=(j == 0), stop=(j == 31))
                        nc.tensor.matmul(dps[:], ones_sb[:], ex[:],
                                         start=(j == 0), stop=(j == 31))
                    rec = asb.tile([128, 512], F32, name="rec", tag="rec")
                    nc.vector.reciprocal(rec[:], dps[:])
                    for ch in range(2):
                        tmp = asb.tile([128, 512], F32, name="tmp", tag="tmp")
                        nc.vector.scalar_tensor_tensor(
                            tmp[:], acc[ch][:], 1.0, rec[:],
                            ALU.mult, ALU.mult)
                        nc.vector.scalar_tensor_tensor(
                            out_sb[:, QH * ch + 512 * ib:QH * ch + 512 * (ib + 1)],
                            tmp[:], consts[:, 9 + ch:10 + ch],
                            x1h[:, QH * ch + 512 * ib:QH * ch + 512 * (ib + 1)],
                            ALU.add, ALU.add)
                    nc.sync.dma_start(
                        out_d[:].rearrange("p (c f) -> p c f",
                                           c=2)[:, :, 512 * ib:512 * (ib + 1)],
                        out_sb[:].rearrange("p (c f) -> p c f",
                                            c=2)[:, :, 512 * ib:512 * (ib + 1)])
    nc.compile()
    return nc


def _prep_shared(inputs):
    f = np.float32
    bf = ml_dtypes.bfloat16

    def bd(w_dw):
        wr = w_dw.reshape(512, 2, 9)
        Wt = np.zeros((4, 128, 9, 128), f)
        m = np.arange(64)
        for k in range(4):
            blk = wr[128 * k:128 * (k + 1)]        # [128, 2, 9]
            for i in range(2):
                for j in range(2):
                    Wt[k, 2 * m + i, :, 2 * m + j] = blk[2 * m + j, i, :]
        return np.ascontiguousarray(Wt.reshape(4, 128, 9 * 128)).astype(bf)

    w1bd = bd(inputs["w1_dw"])
    w2bd = bd(inputs["w2_dw"])

    pw1 = inputs["w1_pw"][:, :, 0, 0]              # [256, 512]
    pw2 = inputs["w2_pw"][:, :, 0, 0]
    pw1T = np.ascontiguousarray(pw1.T.reshape(4, 128, 256)).astype(bf)
    pw2T = np.ascontiguousarray(pw2.T.reshape(4, 128, 256)).astype(bf)

    gamma = float(inputs["gamma"][0])
    wvTg = (inputs["wv"][:, :, 0, 0].T * gamma).reshape(2, 128, 256).astype(bf)
    wkT = inputs["wk"][:, :, 0, 0].T.reshape(2, 128, 32).astype(bf)
    wqT = inputs["wq"][:, :, 0, 0].T.reshape(2, 128, 32).astype(bf)
    projw = np.zeros((128, 640), bf)
    for ch in range(2):
        projw[:, 320 * ch:320 * ch + 256] = wvTg[ch]
        projw[:, 320 * ch + 256:320 * ch + 288] = wkT[ch]
        projw[:, 320 * ch + 288:320 * ch + 320] = wqT[ch]

    def bn_fold(g, b_, mean, var, pw, b_dw, b_pw):
        s = g / np.sqrt(var + EPS)
        bc = pw @ b_dw + b_pw
        t = s * (bc - mean) + b_
        o = np.zeros((128, 4), f)
        o[:, 0], o[:, 1] = s[0:128], t[0:128]
        o[:, 2], o[:, 3] = s[128:256], t[128:256]
        return o

    consts = np.zeros((128, 11), f)
    consts[:, 0:4] = bn_fold(inputs["bn1_g"], inputs["bn1_b"], inputs["bn1_m"],
                             inputs["bn1_v"], pw1, inputs["b1_dw"],
                             inputs["b1_pw"])
    consts[:, 4:8] = bn_fold(inputs["bn2_g"], inputs["bn2_b"], inputs["bn2_m"],
                             inputs["bn2_v"], pw2, inputs["b2_dw"],
                             inputs["b2_pw"])
    consts[0:32, 8] = inputs["bk"]
    consts[32:64, 8] = inputs["bq"]
    consts[:, 9] = gamma * inputs["bv"][0:128]
    consts[:, 10] = gamma * inputs["bv"][128:256]

    return dict(w1bd=w1bd, w2bd=w2bd, pw1T=pw1T, pw2T=pw2T,
                projw=projw, consts=consts)


def _prep_core(inputs, shared, b, h):
    bf = ml_dtypes.bfloat16
    x1 = inputs["x1"][b]          # [256, 64, 64]
    x2 = inputs["x2"][b]
    sub = x1 - x2
    cat1 = np.concatenate([sub, x1], axis=0).reshape(4, 128, 64, 64)
    cat2 = np.concatenate([sub, x2], axis=0).reshape(4, 128, 64, 64)

    def pack(cc, wbd, pwT):
        buf = np.zeros((4, 128, SLOTS, 66), np.float32)
        if h == 0:
            buf[:, :, 1:34, 1:65] = cc[:, :, 0:33, :]
        else:
            buf[:, :, 0:33, 1:65] = cc[:, :, 31:64, :]
        cw = np.zeros((4, 128, CATW), bf)
        cw[:, :, OFF:OFF + SLOTS * PW] = buf.reshape(4, 128, -1)
        cw[:, :, CAT_F:CAT_F + 9 * 128] = wbd
        cw[:, :, CAT_F + 9 * 128:] = pwT
        return cw

    x1r = x1.reshape(256, N)[:, QH * h:QH * (h + 1)]   # [256, QH]
    x1h = np.ascontiguousarray(
        np.concatenate([x1r[0:128], x1r[128:256]], axis=1))  # [128, 2*QH]
    return dict(c1w=pack(cat1, shared["w1bd"], shared["pw1T"]),
                c2w=pack(cat2, shared["w2bd"], shared["pw2T"]),
                x1h=x1h)


def kernel(**inputs):
    if "nc" not in _CACHE:
        _CACHE["nc"] = _build_nc()
    nc = _CACHE["nc"]

    inputs = {k: np.ascontiguousarray(np.asarray(v)) for k, v in inputs.items()}
    shared = _prep_shared(inputs)
    in_maps = []
    for core in range(8):
        b, h = core // 2, core % 2
        m = dict(projw=shared["projw"], consts=shared["consts"])
        m.update(_prep_core(inputs, shared, b, h))
        in_maps.append(m)

    res = run_bass_kernel_spmd(nc, in_maps, list(range(8)))
    out = np.empty((4, 256, N), np.float32)
    for core in range(8):
        b, h = core // 2, core % 2
        r = res.results[core]["out"]
        out[b, 0:128, QH * h:QH * (h + 1)] = r[:, 0:QH]
        out[b, 128:256, QH * h:QH * (h + 1)] = r[:, QH:2 * QH]
    return out.reshape(B, C, H, W)


# revision 13
# speedup vs baseline: 1.2424x; 1.0666x over previous
"""CrossAttention kernel for Trainium2, 8 NeuronCores.

Reference pipeline (B=4, C=256, H=W=64, N=4096, d=C//8=32):
  sub = x1 - x2
  x3 = relu(bn1(pw1(dw1([sub, x1]))))      # dw: 3x3 grouped conv (groups=C)
  x4 = relu(bn2(pw2(dw2([sub, x2]))))      # pw: 1x1 512->256
  q = wq@x4 [B,32,N]; k = wk@x3 [B,32,N]; v = wv@x3 [B,256,N]
  attn = softmax(q^T k);  out = gamma * (v @ attn^T) + x1

Sharding: 8 cores = (batch b) x (pixel-half h). Each core computes BOTH
conv paths only for its own pixel half (with a one-row halo), projects
k / v^T / q from its half, then the pair exchanges k and v^T via a
single AllGather so each core runs flash attention for its 2048 queries
over all 4096 keys.

Device-side choices:
  - dw conv on the PE as 9 accumulating block-diagonal [128x128] matmuls
    over a zero-padded 66-col image layout (8-row windows = 512 output
    columns via a strided rhs AP that skips the pad columns).
  - energy is computed transposed, E^T[j, i], with k/q zero-padded to a
    128-deep contraction so the PE streams at full rate.
  - attention accumulation outputs [channel, query] directly (lhsT =
    v^T block, rhs = exp tile): 512-wide matmuls, no PE transposes. The
    softmax denominator comes from an all-ones lhsT matmul which also
    broadcasts it across partitions for the DVE normalize.
  - all inputs packed into few large DMAs (descriptor generation on the
    sync engine costs ~0.7us per dma_start, so count matters).
  - matmul datapath in bf16 (fp32 PSUM accumulation); normalize /
    residual-add in fp32. gamma folded into wv/bv on the host; bn+biases
    folded into per-channel scale/shift applied during PSUM eviction.
"""

import numpy as np
import ml_dtypes

import concourse.bass as bass
import concourse.mybir as mybir
import concourse.tile as tile
from concourse import bacc
from concourse.bass_utils import run_bass_kernel_spmd

F32 = mybir.dt.float32
BF16 = mybir.dt.bfloat16
F8 = mybir.dt.float8e4
U8 = mybir.dt.uint8
PM = mybir.MatmulPerfMode
AF = mybir.ActivationFunctionType
ALU = mybir.AluOpType

B, C, H, W = 4, 256, 64, 64
N = H * W            # 4096 pixels
QH = N // 2          # pixels per core (queries/own keys)
EPS = 1e-5
PW = 66              # padded row width
OFF = 2              # leading pad elements in padded tiles
SLOTS = 34           # 32 data rows + halo/zero rows
CAT_F = OFF + SLOTS * PW + OFF   # 2248
CATW = CAT_F + 9 * 128 + 256     # cat | dw weights | pw weights = 3656
VT = 272             # v^T row stride: 256 channels + pad (16B-aligned for DoubleRow)
PAIRS = [[0, 1], [2, 3], [4, 5], [6, 7]]

_CACHE = {}


def _build_nc():
    nc = bacc.Bacc("TRN2", target_bir_lowering=False, debug=False, num_devices=8)

    c1w_d = nc.dram_tensor("c1w", [4, 128, CATW], BF16, kind="ExternalInput")
    c2w_d = nc.dram_tensor("c2w", [4, 128, CATW], BF16, kind="ExternalInput")
    x1h_d = nc.dram_tensor("x1h", [128, 2 * QH], F32, kind="ExternalInput")
    consts_d = nc.dram_tensor("consts", [128, 11], F32, kind="ExternalInput")
    projw_d = nc.dram_tensor("projw", [128, 640], BF16, kind="ExternalInput")
    out_d = nc.dram_tensor("out", [128, 2 * QH], F32, kind="ExternalOutput")

    # collective bounce buffers, in bytes: v^T fp8 (128x4128) + k bf16 (32x2048)
    VSZ = 128 * 16 * VT              # v^T bytes
    KVN = VSZ + 32 * QH * 2          # + k bytes = 659456
    kvown_d = nc.dram_tensor("kvown_b", [KVN], U8)
    kvfull_d = nc.dram_tensor("kvfull_b", [2 * KVN], U8)

    with tile.TileContext(nc) as tc:
        with tc.tile_pool(name="persist", bufs=1) as pp:
            catw = [[pp.tile([128, CATW], BF16, name=f"cw{b}_{k}",
                             tag=f"cw{b}_{k}") for k in range(4)]
                    for b in range(2)]
            x3o = [pp.tile([128, QH], BF16, name=f"x3o_{m}", tag=f"x3o_{m}")
                   for m in range(2)]
            x4 = [pp.tile([128, QH], BF16, name=f"x4_{m}", tag=f"x4_{m}")
                  for m in range(2)]
            consts = pp.tile([128, 11], F32, name="consts", tag="consts")
            projw = pp.tile([128, 640], BF16, name="projw", tag="projw")
            k_own = pp.tile([128, QH], BF16, name="k_own", tag="k_own")
            vto = pp.tile([128, 16 * VT], F8, name="vto", tag="vto")
            k_sb = pp.tile([128, N], BF16, name="k_sb", tag="k_sb")
            q_sb = pp.tile([128, QH], BF16, name="q_sb", tag="q_sb")
            vta = pp.tile([128, 32 * VT], F8, name="vta", tag="vta")
            x1h = pp.tile([128, 2 * QH], F32, name="x1h", tag="x1h")
            out_sb = pp.tile([128, 2 * QH], F32, name="osb", tag="osb")
            ones_sb = pp.tile([128, 256], F8, name="ones", tag="ones")
            # zero rows 32:128 so energy matmuls can use full 128-row lhsT
            # (avoids the PE small-tile row-group slowdown)
            nc.gpsimd.memset(k_sb[:], 0.0)
            nc.gpsimd.memset(q_sb[:], 0.0)
            nc.gpsimd.memset(ones_sb[:], 1.0)

            # input DMAs in priority order (conv1, consts, conv2, residual)
            for k in range(4):
                nc.sync.dma_start(catw[0][k][:], c1w_d[k])
            nc.sync.dma_start(consts[:], consts_d[:])
            nc.sync.dma_start(projw[:], projw_d[:])
            for k in range(4):
                nc.sync.dma_start(catw[1][k][:], c2w_d[k])
            nc.sync.dma_start(x1h[:], x1h_d[:])

            def conv_block(cw, bno, xout):
                with tc.tile_pool(name="conv_y", bufs=2) as cyb, \
                     tc.tile_pool(name="conv_ps", bufs=2, space="PSUM") as cps:
                    for w in range(4):
                        y1w = [cyb.tile([128, 512], BF16,
                                        name=f"y1w_{k}", tag=f"y1w_{k}")
                               for k in range(4)]
                        for k in range(4):
                            ps = cps.tile([128, 512], F32, name="dwps", tag="dwps")
                            for t in range(9):
                                dr, dc = t // 3, t % 3
                                start = OFF + (8 * w + dr) * PW + dc - 1
                                rhs = cw[k][:, start:start + 8 * PW] \
                                    .rearrange("p (r c) -> p r c", r=8, c=PW)[:, :, 0:64]
                                nc.tensor.matmul(
                                    ps[:],
                                    cw[k][:, CAT_F + 128 * t:CAT_F + 128 * (t + 1)],
                                    rhs, start=(t == 0), stop=(t == 8))
                            nc.scalar.activation(y1w[k][:], ps[:], AF.Copy)
                        for m in range(2):
                            pp2 = cps.tile([128, 512], F32, name="pwps", tag="pwps")
                            pwo = CAT_F + 9 * 128
                            for k in range(4):
                                nc.tensor.matmul(
                                    pp2[:],
                                    cw[k][:, pwo + 128 * m:pwo + 128 * (m + 1)],
                                    y1w[k][:], start=(k == 0), stop=(k == 3))
                            nc.scalar.activation(
                                xout[m][:, 512 * w:512 * (w + 1)], pp2[:],
                                AF.Relu,
                                bias=consts[:, bno + 2 * m + 1:bno + 2 * m + 2],
                                scale=consts[:, bno + 2 * m:bno + 2 * m + 1])

            conv_block(catw[0], 0, x3o)

            # ---- own-half projections: k_own, v^T_own ----
            with tc.tile_pool(name="proj_ps", bufs=2, space="PSUM") as pps:
                for s in range(4):
                    ps = pps.tile([128, 512], F32, name="kqps", tag="kqps")
                    for ch in range(2):
                        nc.tensor.matmul(ps[0:32, :],
                                         projw[:, 320 * ch + 256:320 * ch + 288],
                                         x3o[ch][:, 512 * s:512 * (s + 1)],
                                         start=(ch == 0), stop=(ch == 1))
                    nc.scalar.activation(k_own[0:32, 512 * s:512 * (s + 1)],
                                         ps[0:32, :], AF.Identity,
                                         bias=consts[0:32, 8:9])
                for j in range(16):
                    ps = pps.tile([128, 256], F32, name="vtps", tag="vtps")
                    for ch in range(2):
                        nc.tensor.matmul(ps[:], x3o[ch][:, 128 * j:128 * (j + 1)],
                                         projw[:, 320 * ch:320 * ch + 256],
                                         start=(ch == 0), stop=(ch == 1))
                    nc.scalar.activation(vto[:, VT * j:VT * j + 256], ps[:], AF.Copy)

                # ship own k / v^T, single AllGather for the pair
                nc.sync.dma_start(
                    kvown_d[0:VSZ].rearrange("(p f) -> p f", p=128),
                    vto[:].bitcast(U8))
                nc.sync.dma_start(
                    kvown_d[VSZ:KVN].rearrange("(p f) -> p f", p=32),
                    k_own[0:32, :].bitcast(U8))
                nc.gpsimd.collective_compute(
                    "AllGather", ALU.bypass, replica_groups=PAIRS,
                    ins=[kvown_d[:].opt()], outs=[kvfull_d[:].opt()])
                for m in range(2):
                    o = m * KVN
                    nc.sync.dma_start(
                        vta[:, 16 * VT * m:16 * VT * (m + 1)].bitcast(U8),
                        kvfull_d[o:o + VSZ].rearrange("(p f) -> p f", p=128))
                    nc.sync.dma_start(
                        k_sb[0:32, QH * m:QH * (m + 1)].bitcast(U8),
                        kvfull_d[o + VSZ:o + KVN].rearrange("(p f) -> p f",
                                                            p=32))

                # conv2 + q overlap with the collective
                conv_block(catw[1], 4, x4)
                for s in range(4):
                    ps = pps.tile([128, 512], F32, name="kqps", tag="kqps")
                    for ch in range(2):
                        nc.tensor.matmul(ps[0:32, :],
                                         projw[:, 320 * ch + 288:320 * ch + 320],
                                         x4[ch][:, 512 * s:512 * (s + 1)],
                                         start=(ch == 0), stop=(ch == 1))
                    nc.scalar.activation(q_sb[0:32, 512 * s:512 * (s + 1)],
                                         ps[0:32, :], AF.Identity,
                                         bias=consts[32:64, 8:9])

            # ---- flash attention (output computed pre-transposed) ----
            with tc.tile_pool(name="att_sb", bufs=2) as asb, \
                 tc.tile_pool(name="acc_ps", bufs=2, space="PSUM") as accp, \
                 tc.tile_pool(name="e_ps", bufs=2, space="PSUM") as epsp:
                for ib in range(4):
                    acc = [accp.tile([128, 512], F32, name=f"acc{c}",
                                     tag=f"acc{c}") for c in range(2)]
                    dps = accp.tile([128, 512], F32, name="dps", tag="dps")
                    eps_t = {}
                    vv = vta[:].rearrange("p (j v) -> p j v", j=32)
                    ones2 = ones_sb[:].rearrange("p (k f) -> p k f", k=2)
                    expair = None
                    for j in range(32):
                        if j == 0:
                            eps_t[0] = epsp.tile([128, 512], F32, name="eps",
                                                 tag="eps")
                            nc.tensor.matmul(eps_t[0][:],
                                             k_sb[:, 0:128],
                                             q_sb[:, 512 * ib:512 * (ib + 1)],
                                             start=True, stop=True)
                        if j % 2 == 0:
                            expair = asb.tile([128, 1024], F8, name="ex",
                                              tag="ex")
                        nc.scalar.activation(
                            expair[:, 512 * (j % 2):512 * (j % 2 + 1)],
                            eps_t[j][:], AF.Exp)
                        if j + 1 < 32:
                            eps_t[j + 1] = epsp.tile([128, 512], F32, name="eps",
                                                     tag="eps")
                            nc.tensor.matmul(eps_t[j + 1][:],
                                             k_sb[:, 128 * (j + 1):128 * (j + 2)],
                                             q_sb[:, 512 * ib:512 * (ib + 1)],
                                             start=True, stop=True)
                        eps_t.pop(j - 1, None)
                        if j % 2 == 1:
                            jp = j // 2
                            rhs2 = expair[:].rearrange("p (k f) -> p k f", k=2)
                            nc.tensor.matmul(acc[0][:],
                                             vv[:, 2 * jp:2 * jp + 2, 0:128],
                                             rhs2, start=(jp == 0),
                                             stop=(jp == 15),
                                             perf_mode=PM.DoubleRow)
                            nc.tensor.matmul(acc[1][:],
                                             vv[:, 2 * jp:2 * jp + 2, 128:256],
                                             rhs2, start=(jp == 0),
                                             stop=(jp == 15),
                                             perf_mode=PM.DoubleRow)
                            nc.tensor.matmul(dps[:], ones2, rhs2,
                                             start=(jp == 0), stop=(jp == 15),
                                             perf_mode=PM.DoubleRow)
                    rec = asb.tile([128, 512], F32, name="rec", tag="rec")
                    nc.vector.reciprocal(rec[:], dps[:])
                    for ch in range(2):
                        tmp = asb.tile([128, 512], F32, name="tmp", tag="tmp")
                        nc.vector.scalar_tensor_tensor(
                            tmp[:], acc[ch][:], 1.0, rec[:],
                            ALU.mult, ALU.mult)
                        nc.vector.scalar_tensor_tensor(
                            out_sb[:, QH * ch + 512 * ib:QH * ch + 512 * (ib + 1)],
                            tmp[:], consts[:, 9 + ch:10 + ch],
                            x1h[:, QH * ch + 512 * ib:QH * ch + 512 * (ib + 1)],
                            ALU.add, ALU.add)
                    nc.sync.dma_start(
                        out_d[:].rearrange("p (c f) -> p c f",
                                           c=2)[:, :, 512 * ib:512 * (ib + 1)],
                        out_sb[:].rearrange("p (c f) -> p c f",
                                            c=2)[:, :, 512 * ib:512 * (ib + 1)])
    nc.compile()
    return nc


def _prep_shared(inputs):
    f = np.float32
    bf = ml_dtypes.bfloat16

    def bd(w_dw):
        wr = w_dw.reshape(512, 2, 9)
        Wt = np.zeros((4, 128, 9, 128), f)
        m = np.arange(64)
        for k in range(4):
            blk = wr[128 * k:128 * (k + 1)]        # [128, 2, 9]
            for i in range(2):
                for j in range(2):
                    Wt[k, 2 * m + i, :, 2 * m + j] = blk[2 * m + j, i, :]
        return np.ascontiguousarray(Wt.reshape(4, 128, 9 * 128)).astype(bf)

    w1bd = bd(inputs["w1_dw"])
    w2bd = bd(inputs["w2_dw"])

    pw1 = inputs["w1_pw"][:, :, 0, 0]              # [256, 512]
    pw2 = inputs["w2_pw"][:, :, 0, 0]
    pw1T = np.ascontiguousarray(pw1.T.reshape(4, 128, 256)).astype(bf)
    pw2T = np.ascontiguousarray(pw2.T.reshape(4, 128, 256)).astype(bf)

    gamma = float(inputs["gamma"][0])
    wvTg = (inputs["wv"][:, :, 0, 0].T * gamma).reshape(2, 128, 256).astype(bf)
    wkT = inputs["wk"][:, :, 0, 0].T.reshape(2, 128, 32).astype(bf)
    wqT = inputs["wq"][:, :, 0, 0].T.reshape(2, 128, 32).astype(bf)
    projw = np.zeros((128, 640), bf)
    for ch in range(2):
        projw[:, 320 * ch:320 * ch + 256] = wvTg[ch]
        projw[:, 320 * ch + 256:320 * ch + 288] = wkT[ch]
        projw[:, 320 * ch + 288:320 * ch + 320] = wqT[ch]

    def bn_fold(g, b_, mean, var, pw, b_dw, b_pw):
        s = g / np.sqrt(var + EPS)
        bc = pw @ b_dw + b_pw
        t = s * (bc - mean) + b_
        o = np.zeros((128, 4), f)
        o[:, 0], o[:, 1] = s[0:128], t[0:128]
        o[:, 2], o[:, 3] = s[128:256], t[128:256]
        return o

    consts = np.zeros((128, 11), f)
    consts[:, 0:4] = bn_fold(inputs["bn1_g"], inputs["bn1_b"], inputs["bn1_m"],
                             inputs["bn1_v"], pw1, inputs["b1_dw"],
                             inputs["b1_pw"])
    consts[:, 4:8] = bn_fold(inputs["bn2_g"], inputs["bn2_b"], inputs["bn2_m"],
                             inputs["bn2_v"], pw2, inputs["b2_dw"],
                             inputs["b2_pw"])
    consts[0:32, 8] = inputs["bk"]
    consts[32:64, 8] = inputs["bq"]
    consts[:, 9] = gamma * inputs["bv"][0:128]
    consts[:, 10] = gamma * inputs["bv"][128:256]

    return dict(w1bd=w1bd, w2bd=w2bd, pw1T=pw1T, pw2T=pw2T,
                projw=projw, consts=consts)


def _prep_core(inputs, shared, b, h):
    bf = ml_dtypes.bfloat16
    x1 = inputs["x1"][b]          # [256, 64, 64]
    x2 = inputs["x2"][b]
    sub = x1 - x2
    cat1 = np.concatenate([sub, x1], axis=0).reshape(4, 128, 64, 64)
    cat2 = np.concatenate([sub, x2], axis=0).reshape(4, 128, 64, 64)

    def pack(cc, wbd, pwT):
        buf = np.zeros((4, 128, SLOTS, 66), np.float32)
        if h == 0:
            buf[:, :, 1:34, 1:65] = cc[:, :, 0:33, :]
        else:
            buf[:, :, 0:33, 1:65] = cc[:, :, 31:64, :]
        cw = np.zeros((4, 128, CATW), bf)
        cw[:, :, OFF:OFF + SLOTS * PW] = buf.reshape(4, 128, -1)
        cw[:, :, CAT_F:CAT_F + 9 * 128] = wbd
        cw[:, :, CAT_F + 9 * 128:] = pwT
        return cw

    x1r = x1.reshape(256, N)[:, QH * h:QH * (h + 1)]   # [256, QH]
    x1h = np.ascontiguousarray(
        np.concatenate([x1r[0:128], x1r[128:256]], axis=1))  # [128, 2*QH]
    return dict(c1w=pack(cat1, shared["w1bd"], shared["pw1T"]),
                c2w=pack(cat2, shared["w2bd"], shared["pw2T"]),
                x1h=x1h)


def kernel(**inputs):
    if "nc" not in _CACHE:
        _CACHE["nc"] = _build_nc()
    nc = _CACHE["nc"]

    inputs = {k: np.ascontiguousarray(np.asarray(v)) for k, v in inputs.items()}
    shared = _prep_shared(inputs)
    in_maps = []
    for core in range(8):
        b, h = core // 2, core % 2
        m = dict(projw=shared["projw"], consts=shared["consts"])
        m.update(_prep_core(inputs, shared, b, h))
        in_maps.append(m)

    res = run_bass_kernel_spmd(nc, in_maps, list(range(8)))
    out = np.empty((4, 256, N), np.float32)
    for core in range(8):
        b, h = core // 2, core % 2
        r = res.results[core]["out"]
        out[b, 0:128, QH * h:QH * (h + 1)] = r[:, 0:QH]
        out[b, 128:256, QH * h:QH * (h + 1)] = r[:, QH:2 * QH]
    return out.reshape(B, C, H, W)


# revision 14
# speedup vs baseline: 1.3755x; 1.1071x over previous
"""CrossAttention kernel for Trainium2, 8 NeuronCores.

Reference pipeline (B=4, C=256, H=W=64, N=4096, d=C//8=32):
  sub = x1 - x2
  x3 = relu(bn1(pw1(dw1([sub, x1]))))      # dw: 3x3 grouped conv (groups=C)
  x4 = relu(bn2(pw2(dw2([sub, x2]))))      # pw: 1x1 512->256
  q = wq@x4 [B,32,N]; k = wk@x3 [B,32,N]; v = wv@x3 [B,256,N]
  attn = softmax(q^T k);  out = gamma * (v @ attn^T) + x1

Sharding: 8 cores = (batch b) x (pixel-half h). Each core computes BOTH
conv paths only for its own pixel half (with a one-row halo), projects
k / v^T / q from its half, then the pair exchanges k and v^T via a
single AllGather so each core runs flash attention for its 2048 queries
over all 4096 keys.

Device-side choices:
  - dw conv on the PE as 9 accumulating block-diagonal [128x128] matmuls
    over a zero-padded 66-col image layout (8-row windows = 512 output
    columns via a strided rhs AP that skips the pad columns).
  - energy is computed transposed, E^T[j, i], with k/q zero-padded to a
    128-deep contraction so the PE streams at full rate.
  - attention accumulation outputs [channel, query] directly (lhsT =
    v^T block, rhs = exp tile): 512-wide matmuls, no PE transposes. The
    softmax denominator comes from an all-ones lhsT matmul which also
    broadcasts it across partitions for the DVE normalize.
  - all inputs packed into few large DMAs (descriptor generation on the
    sync engine costs ~0.7us per dma_start, so count matters).
  - matmul datapath in bf16 (fp32 PSUM accumulation); normalize /
    residual-add in fp32. gamma folded into wv/bv on the host; bn+biases
    folded into per-channel scale/shift applied during PSUM eviction.
"""

import numpy as np
import ml_dtypes

import concourse.bass as bass
import concourse.mybir as mybir
import concourse.tile as tile
from concourse import bacc
from concourse.bass_utils import run_bass_kernel_spmd

F32 = mybir.dt.float32
BF16 = mybir.dt.bfloat16
F8 = mybir.dt.float8e4
U8 = mybir.dt.uint8
PM = mybir.MatmulPerfMode
AF = mybir.ActivationFunctionType
ALU = mybir.AluOpType

B, C, H, W = 4, 256, 64, 64
N = H * W            # 4096 pixels
QH = N // 2          # pixels per core (queries/own keys)
EPS = 1e-5
PW = 66              # padded row width
OFF = 2              # leading pad elements in padded tiles
SLOTS = 34           # 32 data rows + halo/zero rows
CAT_F = OFF + SLOTS * PW + OFF   # 2248
CATW = CAT_F + 9 * 128 + 256     # cat | dw weights | pw weights = 3656
VT = 272             # v^T row stride: 256 channels + pad (16B-aligned for DoubleRow)
PAIRS = [[0, 1], [2, 3], [4, 5], [6, 7]]

_CACHE = {}


def _build_nc():
    nc = bacc.Bacc("TRN2", target_bir_lowering=False, debug=False, num_devices=8)

    c1w_d = nc.dram_tensor("c1w", [4, 128, CATW], BF16, kind="ExternalInput")
    c2w_d = nc.dram_tensor("c2w", [4, 128, CATW], BF16, kind="ExternalInput")
    x1h_d = nc.dram_tensor("x1h", [128, 2 * QH], F32, kind="ExternalInput")
    consts_d = nc.dram_tensor("consts", [128, 11], F32, kind="ExternalInput")
    projw_d = nc.dram_tensor("projw", [128, 640], BF16, kind="ExternalInput")
    out_d = nc.dram_tensor("out", [128, 2 * QH], F32, kind="ExternalOutput")

    # collective bounce buffers, in bytes: v^T fp8 (128x4128) + k bf16 (32x2048)
    VSZ = 128 * 16 * VT              # v^T bytes
    KVN = VSZ + 32 * QH * 2          # + k bytes = 659456
    kvown_d = nc.dram_tensor("kvown_b", [KVN], U8)
    kvfull_d = nc.dram_tensor("kvfull_b", [2 * KVN], U8)

    with tile.TileContext(nc) as tc:
        with tc.tile_pool(name="persist", bufs=1) as pp:
            catw = [[pp.tile([128, CATW], BF16, name=f"cw{b}_{k}",
                             tag=f"cw{b}_{k}") for k in range(4)]
                    for b in range(2)]
            x3o = [pp.tile([128, QH], BF16, name=f"x3o_{m}", tag=f"x3o_{m}")
                   for m in range(2)]
            x4 = [pp.tile([128, QH], BF16, name=f"x4_{m}", tag=f"x4_{m}")
                  for m in range(2)]
            consts = pp.tile([128, 11], F32, name="consts", tag="consts")
            projw = pp.tile([128, 640], BF16, name="projw", tag="projw")
            k_own = pp.tile([128, QH], BF16, name="k_own", tag="k_own")
            vto = pp.tile([128, 16 * VT], F8, name="vto", tag="vto")
            k_sb = pp.tile([128, N], BF16, name="k_sb", tag="k_sb")
            q_sb = pp.tile([128, QH], BF16, name="q_sb", tag="q_sb")
            vta = pp.tile([128, 32 * VT], F8, name="vta", tag="vta")
            x1h = pp.tile([128, 2 * QH], F32, name="x1h", tag="x1h")
            out_sb = pp.tile([128, 2 * QH], F32, name="osb", tag="osb")
            ones_sb = pp.tile([128, 256], F8, name="ones", tag="ones")
            # zero rows 32:128 so energy matmuls can use full 128-row lhsT
            # (avoids the PE small-tile row-group slowdown)
            nc.gpsimd.memset(k_sb[:], 0.0)
            nc.gpsimd.memset(q_sb[:], 0.0)
            nc.gpsimd.memset(ones_sb[:], 1.0)

            # input DMAs in priority order (conv1, consts, conv2, residual)
            for k in range(4):
                nc.sync.dma_start(catw[0][k][:], c1w_d[k])
            nc.sync.dma_start(consts[:], consts_d[:])
            nc.sync.dma_start(projw[:], projw_d[:])
            for k in range(4):
                nc.sync.dma_start(catw[1][k][:], c2w_d[k])
            nc.sync.dma_start(x1h[:], x1h_d[:])

            def conv_block(cw, bno, xout):
                with tc.tile_pool(name="conv_y", bufs=2) as cyb, \
                     tc.tile_pool(name="conv_ps", bufs=2, space="PSUM") as cps:
                    for w in range(4):
                        y1w = [cyb.tile([128, 512], BF16,
                                        name=f"y1w_{k}", tag=f"y1w_{k}")
                               for k in range(4)]
                        for k in range(4):
                            ps = cps.tile([128, 512], F32, name="dwps", tag="dwps")
                            for t in range(9):
                                dr, dc = t // 3, t % 3
                                start = OFF + (8 * w + dr) * PW + dc - 1
                                rhs = cw[k][:, start:start + 8 * PW] \
                                    .rearrange("p (r c) -> p r c", r=8, c=PW)[:, :, 0:64]
                                nc.tensor.matmul(
                                    ps[:],
                                    cw[k][:, CAT_F + 128 * t:CAT_F + 128 * (t + 1)],
                                    rhs, start=(t == 0), stop=(t == 8))
                            nc.scalar.activation(y1w[k][:], ps[:], AF.Copy)
                        for m in range(2):
                            pp2 = cps.tile([128, 512], F32, name="pwps", tag="pwps")
                            pwo = CAT_F + 9 * 128
                            for k in range(4):
                                nc.tensor.matmul(
                                    pp2[:],
                                    cw[k][:, pwo + 128 * m:pwo + 128 * (m + 1)],
                                    y1w[k][:], start=(k == 0), stop=(k == 3))
                            nc.scalar.activation(
                                xout[m][:, 512 * w:512 * (w + 1)], pp2[:],
                                AF.Relu,
                                bias=consts[:, bno + 2 * m + 1:bno + 2 * m + 2],
                                scale=consts[:, bno + 2 * m:bno + 2 * m + 1])

            conv_block(catw[0], 0, x3o)

            # ---- own-half projections: k_own, v^T_own ----
            with tc.tile_pool(name="proj_ps", bufs=2, space="PSUM") as pps:
                for s in range(4):
                    ps = pps.tile([128, 512], F32, name="kqps", tag="kqps")
                    for ch in range(2):
                        nc.tensor.matmul(ps[0:32, :],
                                         projw[:, 320 * ch + 256:320 * ch + 288],
                                         x3o[ch][:, 512 * s:512 * (s + 1)],
                                         start=(ch == 0), stop=(ch == 1))
                    nc.scalar.activation(k_own[0:32, 512 * s:512 * (s + 1)],
                                         ps[0:32, :], AF.Identity,
                                         bias=consts[0:32, 8:9])
                for j in range(16):
                    ps = pps.tile([128, 256], F32, name="vtps", tag="vtps")
                    for ch in range(2):
                        nc.tensor.matmul(ps[:], x3o[ch][:, 128 * j:128 * (j + 1)],
                                         projw[:, 320 * ch:320 * ch + 256],
                                         start=(ch == 0), stop=(ch == 1))
                    nc.scalar.activation(vto[:, VT * j:VT * j + 256], ps[:], AF.Copy)

                # ship own k / v^T, single AllGather for the pair
                nc.sync.dma_start(
                    kvown_d[0:VSZ].rearrange("(p f) -> p f", p=128),
                    vto[:].bitcast(U8))
                nc.sync.dma_start(
                    kvown_d[VSZ:KVN].rearrange("(p f) -> p f", p=32),
                    k_own[0:32, :].bitcast(U8))
                nc.gpsimd.collective_compute(
                    "AllGather", ALU.bypass, replica_groups=PAIRS,
                    ins=[kvown_d[:].opt()], outs=[kvfull_d[:].opt()])
                for m in range(2):
                    o = m * KVN
                    nc.sync.dma_start(
                        vta[:, 16 * VT * m:16 * VT * (m + 1)].bitcast(U8),
                        kvfull_d[o:o + VSZ].rearrange("(p f) -> p f", p=128))
                    nc.sync.dma_start(
                        k_sb[0:32, QH * m:QH * (m + 1)].bitcast(U8),
                        kvfull_d[o + VSZ:o + KVN].rearrange("(p f) -> p f",
                                                            p=32))

                # conv2 + q overlap with the collective
                conv_block(catw[1], 4, x4)
                for s in range(4):
                    ps = pps.tile([128, 512], F32, name="kqps", tag="kqps")
                    for ch in range(2):
                        nc.tensor.matmul(ps[0:32, :],
                                         projw[:, 320 * ch + 288:320 * ch + 320],
                                         x4[ch][:, 512 * s:512 * (s + 1)],
                                         start=(ch == 0), stop=(ch == 1))
                    nc.scalar.activation(q_sb[0:32, 512 * s:512 * (s + 1)],
                                         ps[0:32, :], AF.Identity,
                                         bias=consts[32:64, 8:9])

            # ---- flash attention (output computed pre-transposed) ----
            with tc.tile_pool(name="att_sb", bufs=2) as asb, \
                 tc.tile_pool(name="acc_ps", bufs=1, space="PSUM") as accp, \
                 tc.tile_pool(name="e_ps", bufs=2, space="PSUM") as epsp:
                for ib in range(4):
                    acc = [accp.tile([128, 512], F32, name=f"acc{c}",
                                     tag=f"acc{c}") for c in range(2)]
                    dps = accp.tile([128, 512], F32, name="dps", tag="dps")
                    eps_t = {}
                    vv = vta[:].rearrange("p (j v) -> p j v", j=32)
                    ones2 = ones_sb[:].rearrange("p (k f) -> p k f", k=2)

                    def energy_pair(p):
                        eps_t[p] = epsp.tile([128, 1024], F32, name="eps",
                                             tag="eps")
                        for h in range(2):
                            nc.tensor.matmul(
                                eps_t[p][:, 512 * h:512 * (h + 1)],
                                k_sb[:, 128 * (2 * p + h):128 * (2 * p + h + 1)],
                                q_sb[:, 512 * ib:512 * (ib + 1)],
                                start=True, stop=True)

                    for p in range(16):
                        if p == 0:
                            energy_pair(0)
                        expair = asb.tile([128, 1024], F8, name="ex", tag="ex")
                        nc.scalar.activation(expair[:], eps_t[p][:], AF.Exp)
                        if p + 1 < 16:
                            energy_pair(p + 1)
                        eps_t.pop(p - 1, None)
                        rhs2 = expair[:].rearrange("p (k f) -> p k f", k=2)
                        nc.tensor.matmul(acc[0][:],
                                         vv[:, 2 * p:2 * p + 2, 0:128],
                                         rhs2, start=(p == 0), stop=(p == 15),
                                         perf_mode=PM.DoubleRow)
                        nc.tensor.matmul(acc[1][:],
                                         vv[:, 2 * p:2 * p + 2, 128:256],
                                         rhs2, start=(p == 0), stop=(p == 15),
                                         perf_mode=PM.DoubleRow)
                        nc.tensor.matmul(dps[:], ones2, rhs2,
                                         start=(p == 0), stop=(p == 15),
                                         perf_mode=PM.DoubleRow)
                    rec = asb.tile([128, 512], F32, name="rec", tag="rec")
                    nc.vector.reciprocal(rec[:], dps[:])
                    for ch in range(2):
                        tmp = asb.tile([128, 512], F32, name="tmp", tag="tmp")
                        nc.vector.scalar_tensor_tensor(
                            tmp[:], acc[ch][:], 1.0, rec[:],
                            ALU.mult, ALU.mult)
                        nc.vector.scalar_tensor_tensor(
                            out_sb[:, QH * ch + 512 * ib:QH * ch + 512 * (ib + 1)],
                            tmp[:], consts[:, 9 + ch:10 + ch],
                            x1h[:, QH * ch + 512 * ib:QH * ch + 512 * (ib + 1)],
                            ALU.add, ALU.add)
                    nc.sync.dma_start(
                        out_d[:].rearrange("p (c f) -> p c f",
                                           c=2)[:, :, 512 * ib:512 * (ib + 1)],
                        out_sb[:].rearrange("p (c f) -> p c f",
                                            c=2)[:, :, 512 * ib:512 * (ib + 1)])
    nc.compile()
    return nc


def _prep_shared(inputs):
    f = np.float32
    bf = ml_dtypes.bfloat16

    def bd(w_dw):
        wr = w_dw.reshape(512, 2, 9)
        Wt = np.zeros((4, 128, 9, 128), f)
        m = np.arange(64)
        for k in range(4):
            blk = wr[128 * k:128 * (k + 1)]        # [128, 2, 9]
            for i in range(2):
                for j in range(2):
                    Wt[k, 2 * m + i, :, 2 * m + j] = blk[2 * m + j, i, :]
        return np.ascontiguousarray(Wt.reshape(4, 128, 9 * 128)).astype(bf)

    w1bd = bd(inputs["w1_dw"])
    w2bd = bd(inputs["w2_dw"])

    pw1 = inputs["w1_pw"][:, :, 0, 0]              # [256, 512]
    pw2 = inputs["w2_pw"][:, :, 0, 0]
    pw1T = np.ascontiguousarray(pw1.T.reshape(4, 128, 256)).astype(bf)
    pw2T = np.ascontiguousarray(pw2.T.reshape(4, 128, 256)).astype(bf)

    gamma = float(inputs["gamma"][0])
    wvTg = (inputs["wv"][:, :, 0, 0].T * gamma).reshape(2, 128, 256).astype(bf)
    wkT = inputs["wk"][:, :, 0, 0].T.reshape(2, 128, 32).astype(bf)
    wqT = inputs["wq"][:, :, 0, 0].T.reshape(2, 128, 32).astype(bf)
    projw = np.zeros((128, 640), bf)
    for ch in range(2):
        projw[:, 320 * ch:320 * ch + 256] = wvTg[ch]
        projw[:, 320 * ch + 256:320 * ch + 288] = wkT[ch]
        projw[:, 320 * ch + 288:320 * ch + 320] = wqT[ch]

    def bn_fold(g, b_, mean, var, pw, b_dw, b_pw):
        s = g / np.sqrt(var + EPS)
        bc = pw @ b_dw + b_pw
        t = s * (bc - mean) + b_
        o = np.zeros((128, 4), f)
        o[:, 0], o[:, 1] = s[0:128], t[0:128]
        o[:, 2], o[:, 3] = s[128:256], t[128:256]
        return o

    consts = np.zeros((128, 11), f)
    consts[:, 0:4] = bn_fold(inputs["bn1_g"], inputs["bn1_b"], inputs["bn1_m"],
                             inputs["bn1_v"], pw1, inputs["b1_dw"],
                             inputs["b1_pw"])
    consts[:, 4:8] = bn_fold(inputs["bn2_g"], inputs["bn2_b"], inputs["bn2_m"],
                             inputs["bn2_v"], pw2, inputs["b2_dw"],
                             inputs["b2_pw"])
    consts[0:32, 8] = inputs["bk"]
    consts[32:64, 8] = inputs["bq"]
    consts[:, 9] = gamma * inputs["bv"][0:128]
    consts[:, 10] = gamma * inputs["bv"][128:256]

    return dict(w1bd=w1bd, w2bd=w2bd, pw1T=pw1T, pw2T=pw2T,
                projw=projw, consts=consts)


def _prep_core(inputs, shared, b, h):
    bf = ml_dtypes.bfloat16
    x1 = inputs["x1"][b]          # [256, 64, 64]
    x2 = inputs["x2"][b]
    sub = x1 - x2
    cat1 = np.concatenate([sub, x1], axis=0).reshape(4, 128, 64, 64)
    cat2 = np.concatenate([sub, x2], axis=0).reshape(4, 128, 64, 64)

    def pack(cc, wbd, pwT):
        buf = np.zeros((4, 128, SLOTS, 66), np.float32)
        if h == 0:
            buf[:, :, 1:34, 1:65] = cc[:, :, 0:33, :]
        else:
            buf[:, :, 0:33, 1:65] = cc[:, :, 31:64, :]
        cw = np.zeros((4, 128, CATW), bf)
        cw[:, :, OFF:OFF + SLOTS * PW] = buf.reshape(4, 128, -1)
        cw[:, :, CAT_F:CAT_F + 9 * 128] = wbd
        cw[:, :, CAT_F + 9 * 128:] = pwT
        return cw

    x1r = x1.reshape(256, N)[:, QH * h:QH * (h + 1)]   # [256, QH]
    x1h = np.ascontiguousarray(
        np.concatenate([x1r[0:128], x1r[128:256]], axis=1))  # [128, 2*QH]
    return dict(c1w=pack(cat1, shared["w1bd"], shared["pw1T"]),
                c2w=pack(cat2, shared["w2bd"], shared["pw2T"]),
                x1h=x1h)


def kernel(**inputs):
    if "nc" not in _CACHE:
        _CACHE["nc"] = _build_nc()
    nc = _CACHE["nc"]

    inputs = {k: np.ascontiguousarray(np.asarray(v)) for k, v in inputs.items()}
    shared = _prep_shared(inputs)
    in_maps = []
    for core in range(8):
        b, h = core // 2, core % 2
        m = dict(projw=shared["projw"], consts=shared["consts"])
        m.update(_prep_core(inputs, shared, b, h))
        in_maps.append(m)

    res = run_bass_kernel_spmd(nc, in_maps, list(range(8)))
    out = np.empty((4, 256, N), np.float32)
    for core in range(8):
        b, h = core // 2, core % 2
        r = res.results[core]["out"]
        out[b, 0:128, QH * h:QH * (h + 1)] = r[:, 0:QH]
        out[b, 128:256, QH * h:QH * (h + 1)] = r[:, QH:2 * QH]
    return out.reshape(B, C, H, W)
